# revision 38
# baseline (speedup 1.0000x reference)
"""Trainium2 Bass kernel for nn_CrossAttention (B=2,N=6,D=128,M=625,H=28,W=60, 4 heads x 32).

Sharding: 8 cores = 2 batches x 4 query-token shards. Zero collectives.

v2 restructure (from the 590us baseline):
- attention j-loop de-serialized: the baseline's critical cycle was
  EXP(j) -> AV(j) -> score(j+1) -> EXP(j+1) at 2.65us/iter because PSUM
  had no room to double-buffer score tiles. New PSUM map (8 banks exact):
  sca (heads 0,1; 2 banks x 2 bufs) + sc2 + sc3 (1 bank, single-buffered,
  fine-grained DVE chains) + av (1 bank: 4 heads x 32 col-tiled, no Z
  column) + zden (1 bank: softmax denominators via ones-stationary
  matmuls at partitions 0/32/64/96).
- AV/Z matmuls delayed by 2 ktiles behind the scores so the PE never
  stalls on the activations; probabilities triple-buffered in SBUF.
- softmax normalize: strided-partition gather of the 4 Z rows -> one
  reciprocal -> one PE broadcast matmul (head-map stationary) -> one
  tensor_mul. Replaces the 26us of 1-partition reciprocals.
- proj is one 128x128 matmul per q-tile (head-major aT layout).
Everything else (LN folds, bf16 operands, split exact/bit-trick exp)
as in the baseline.
"""

import numpy as np
import ml_dtypes

import concourse.bass as bass
import concourse.mybir as mybir
import concourse.tile as tile
from concourse import bass_utils
from concourse.vector_clock import ScopedClock, VectorClock
from concourse.tile_scheduler import N_PROCS

F32 = mybir.dt.float32
BF16 = mybir.dt.bfloat16
I16 = mybir.dt.int16
AF = mybir.ActivationFunctionType
OP = mybir.AluOpType

B, N, D, M, H, W = 2, 6, 128, 625, 28, 60
HEADS, DHEAD = 4, 32
NQ_FULL = N * M            # 3750
NK = N * H * W             # 10080
TQ = 938                   # padded per-core query shard
EPS = 1e-5
EXP_C = 184.66496          # 128 * log2(e): bf16-bits exp trick multiplier
EXP_B = 16256.0            # 127 * 128: bf16 exponent bias in bit space

KT = 128
N_KT = (NK + KT - 1) // KT          # 79 (last = 96)
N_QT = (TQ + KT - 1) // KT          # 8  (last = 42)
Q_CHUNKS = [(0, 512), (512, TQ - 512)]
KC = 512
K_CHUNKS = [(o, min(KC, NK - o)) for o in range(0, NK, KC)]   # 20 (last = 352)
AV_DELAY = 2


def _k_tiles():
    for j in range(N_KT):
        off = j * KT
        yield j, off, min(KT, NK - off)


def _q_tiles():
    for j in range(N_QT):
        off = j * KT
        yield j, off, min(KT, TQ - off)


def _split_multiwait_json(bir_json: bytes) -> bytes:
    """This walrus build allows only one sync-wait per instruction: move
    extra on_wait entries onto EventSemaphore instructions inserted just
    before the owner (same engine, so ordering is preserved)."""
    import json
    bir = json.loads(bir_json)
    for fn in bir["functions"]:
        for blk in fn["blocks"]:
            out = []
            for ins in blk["instructions"]:
                si = ins.get("sync_info")
                waits = (si or {}).get("on_wait") or []
                if len(waits) > 1:
                    for wi, w in enumerate(waits[:-1]):
                        out.append({
                            "debug": ins.get("debug", 0),
                            "engine": ins["engine"],
                            "ins": [], "outs": [],
                            "name": f"{ins['name']}-xw{wi}",
                            "opcode": "EventSemaphore",
                            "sync_info": {"on_update": [], "on_wait": [w]},
                        })
                    si["on_wait"] = [waits[-1]]
                out.append(ins)
            blk["instructions"] = out
    return json.dumps(bir).encode()


def _install_compile_patch():
    from concourse import bass_utils as bu
    if getattr(bu, "_mw_patched", False):
        return
    orig = bu.compile_bir_kernel

    def patched(bir_json, tmpdir, neff_name="file.neff"):
        return orig(_split_multiwait_json(bir_json), tmpdir, neff_name)

    bu.compile_bir_kernel = patched
    bu._mw_patched = True
    try:
        from concourse import bass2jax
        if getattr(bass2jax, "compile_bir_kernel", None) is orig:
            bass2jax.compile_bir_kernel = patched
    except ImportError:
        pass


class _SplitDrainTileContext(tile.TileContext):
    """This walrus build rejects >1 sem wait on a Drain; split the exit
    drain's waits across per-proc drains (one wait each)."""

    def _drain_and_barrier(self, tick_clock, wait_clock):
        full = tick_clock.global_clock
        for p in range(N_PROCS):
            mask = VectorClock([(1 << 30) if i == p else 0 for i in range(N_PROCS)])
            partial = full.copy()
            partial.elementwise_min(mask)
            d = self.nc.sync.drain()
            wait_clock.add_sem_waits(d.ins, ScopedClock({None: partial}))
        self.nc.all_engine_barrier()
        assert self.sems is not None
        popped = self.nc._tile_sem_poison_stack.pop()
        assert popped is self._sem_poison
        self.nc.clear_and_free_semaphores(list(self.sems.allocated().values()))
        self.nc.all_engine_barrier()


def _var_alpha(nc, wp, var, n_tiles, al, eps_ap, al184=None):
    """al = rsqrt(var+eps) from a compact [128, n] variance tile."""
    sd = wp.tile([128, n_tiles], F32, tag="vasd")
    nc.scalar.activation(sd[:, :], var[:, :], AF.Sqrt, bias=eps_ap)
    nc.vector.reciprocal(al[:, :], sd[:, :])
    if al184 is not None:
        nc.vector.tensor_scalar(al184[:, :], al[:, :], EXP_C, None, OP.mult)


def _ln_alpha(nc, wp, mv, n_tiles, al, eps_ap, al184=None, nmu=None):
    """From interleaved bn_aggr stats mv [128, 2*n]: al = rsqrt(var+eps),
    optionally al184 = al*EXP_C and nmu = -mean."""
    mvv = mv[:, :].rearrange("p (t two) -> p t two", two=2)
    var_ap = mvv[:, :, 1:2].rearrange("p t o -> p (t o)")
    sd = wp.tile([128, n_tiles], F32, tag="lnsd")
    nc.scalar.activation(sd[:, :], var_ap, AF.Sqrt, bias=eps_ap)
    nc.vector.reciprocal(al[:, :], sd[:, :])
    if al184 is not None:
        nc.vector.tensor_scalar(al184[:, :], al[:, :], EXP_C, None, OP.mult)
    if nmu is not None:
        nc.vector.tensor_scalar(
            nmu[:, :], mvv[:, :, 0:1].rearrange("p t o -> p (t o)"),
            -1.0, None, OP.mult)


def build_program(host):
    nc = bass.Bass()

    def inp(name, shape, dt=BF16):
        return nc.dram_tensor(name, list(shape), dt, kind="ExternalInput")

    xq = inp("xq", (128, TQ))
    xk = inp("xk", (128, NK))
    xv = inp("xv", (128, NK))
    xskip = inp("xskip", (128, N_QT * 128), F32)
    wqc = inp("wqc", (128, 128))
    wkc = inp("wkc", (128, 128))
    wvc = inp("wvc", (128, 128))
    pjA = inp("pjA", (128, 128))
    pjB = inp("pjB", (128, 128))
    zhmA = inp("zhmA", (128, 128))
    zhmB = inp("zhmB", (128, 128))
    w1a = inp("w1a", (128, 128))
    w1b = inp("w1b", (128, 128))
    w2a = inp("w2a", (128, 128))
    w2b = inp("w2b", (128, 128))
    id16 = inp("id16", (128, 128))
    id32 = inp("id32", (128, 128), F32)
    ones16 = inp("ones16", (128, 128))
    y = nc.dram_tensor("y", [128, TQ], F32, kind="ExternalOutput")

    has_bq = host["has_bq"]
    has_b1 = host["has_b1"]
    has_b2 = host["has_b2"]
    has_post = host["has_post"]
    if has_bq:
        bqcol = inp("bqcol", (128, 1), F32)
    if has_b1:
        b1acol = inp("b1acol", (128, 1), F32)
        b1bcol = inp("b1bcol", (128, 1), F32)
    if has_b2:
        b2row = inp("b2row", (1, 128))
    if has_post:
        pogb = inp("pogb", (128, 128), F32)
        pobb = inp("pobb", (128, 128), F32)

    with _SplitDrainTileContext(nc) as tc:
        import contextlib
        with contextlib.ExitStack() as ctx:
            cpool = ctx.enter_context(tc.tile_pool(name="consts", bufs=1))
            big = ctx.enter_context(tc.tile_pool(name="big", bufs=1))

            def load_const(t, shape, dt=BF16):
                s = cpool.tile(list(shape), dt, tag=t.name)
                nc.sync.dma_start(out=s[:], in_=t[:])
                return s

            wqc_s = load_const(wqc, (128, 128))
            wkc_s = load_const(wkc, (128, 128))
            wvc_s = load_const(wvc, (128, 128))
            pjA_s = load_const(pjA, (128, 128))
            pjB_s = load_const(pjB, (128, 128))
            zhmA_s = load_const(zhmA, (128, 128))
            zhmB_s = load_const(zhmB, (128, 128))
            w1a_s = load_const(w1a, (128, 128))
            w1b_s = load_const(w1b, (128, 128))
            w2a_s = load_const(w2a, (128, 128))
            w2b_s = load_const(w2b, (128, 128))
            id16_s = load_const(id16, (128, 128))
            id32_s = load_const(id32, (128, 128), F32)
            ones16_s = load_const(ones16, (128, 128))
            bq_s = load_const(bqcol, (128, 1), F32) if has_bq else None
            b1a_s = load_const(b1acol, (128, 1), F32) if has_b1 else None
            b1b_s = load_const(b1bcol, (128, 1), F32) if has_b1 else None
            b2_s = load_const(b2row, (1, 128)) if has_b2 else None
            if has_post:
                pog_s = load_const(pogb, (128, 128), F32)
                pob_s = load_const(pobb, (128, 128), F32)

            eps_s = cpool.tile([128, 1], F32, tag="eps")
            nc.vector.memset(eps_s[:, :], EPS)

            xq_sb = big.tile([128, TQ], BF16, tag="xq_sb")
            xk_sb = big.tile([128, NK], BF16, tag="xk_sb")
            xv_sb = big.tile([128, NK], BF16, tag="xv_sb")
            skip_sb = big.tile([128, N_QT * 128], F32, tag="skip_sb")
            khT = big.tile([128, NK], BF16, tag="khT")
            qhT = big.tile([128, TQ], BF16, tag="qhT")
            # per ktile: 4 heads x (32 V dims + 1/alpha_v column) -> the AV
            # matmul also produces the softmax denominator at rows 32/96.
            vpack = big.tile([128, N_KT * 132], BF16, tag="vpack")
            aTA = big.tile([128, TQ], BF16, tag="aTA")
            aTB = big.tile([128, TQ], BF16, tag="aTB")
            z_sb = big.tile([128, N_QT * 128], F32, tag="z_sb")
            outfm = big.tile([128, TQ], F32, tag="outfm")
            qn_sb = big.tile([128, N_QT * 128], BF16, tag="qn_sb")
            qn_fm = big.tile([128, TQ], BF16, tag="qn_fm")
            alK = big.tile([128, N_KT], F32, tag="alK")
            al184K = big.tile([128, N_KT], F32, tag="al184K")
            alV = big.tile([128, N_KT], F32, tag="alV")
            lnalv = big.tile([128, N_KT], F32, tag="lnalv")
            b184 = big.tile([128, N_KT], F32, tag="b184")
            invav = big.tile([128, N_KT], BF16, tag="invav")
            b6K = big.tile([128, N_KT, 6], F32, tag="b6K")
            b6V = big.tile([128, N_KT, 6], F32, tag="b6V")
            varK = big.tile([128, N_KT], F32, tag="varK")
            varV = big.tile([128, N_KT], F32, tag="varV")
            alQ = big.tile([128, N_QT], F32, tag="alQ")
            mvQ = big.tile([128, 2 * N_QT], F32, tag="mvQ")

            nc.sync.dma_start(out=xq_sb[:], in_=xq[:])
            # chunked so the first transposes start after ~1/4 of the load
            for do in range(0, NK, 2560):
                dn = min(2560, NK - do)
                nc.sync.dma_start(out=xk_sb[:, do:do + dn], in_=xk[:, do:do + dn])
                nc.sync.dma_start(out=xv_sb[:, do:do + dn], in_=xv[:, do:do + dn])
            nc.sync.dma_start(out=skip_sb[:], in_=xskip[:])

            # proj reads all 128 rows of aTA/aTB; rows 33-63/97-127 are never
            # written (zero weights in pjA/pjB) — zero them so junk can't NaN
            nc.vector.memset(aTA[:, :], 0.0)
            nc.vector.memset(aTB[:, :], 0.0)

            # ---------------- Q: full LN (token-major) + projection ----------
            with contextlib.ExitStack() as qctx:
                qps = qctx.enter_context(tc.tile_pool(name="q_ps", bufs=1, space="PSUM"))
                qtr = qctx.enter_context(tc.tile_pool(name="q_tr", bufs=2, space="PSUM"))
                qpj = qctx.enter_context(tc.tile_pool(name="q_pj", bufs=2, space="PSUM"))
                qwp = qctx.enter_context(tc.tile_pool(name="q_wp", bufs=3))

                qT = qps.tile([128, N_QT, 128], BF16, tag="qT")
                for j, off, tsz in _q_tiles():
                    nc.tensor.matmul(qT[0:tsz, j, :], xq_sb[:, off:off + tsz],
                                     id16_s[:, :], is_transpose=True,
                                     start=True, stop=True)
                for j, off, tsz in _q_tiles():
                    b6 = qwp.tile([128, 6], F32, tag="qb6")
                    nc.vector.bn_stats(b6[0:tsz, :], qT[0:tsz, j, :])
                    nc.vector.bn_aggr(mvQ[0:tsz, 2 * j:2 * j + 2], b6[0:tsz, :])
                _ln_alpha(nc, qwp, mvQ, N_QT, alQ, eps_s[:, 0:1])
                for j, off, tsz in _q_tiles():
                    nc.vector.tensor_scalar(qn_sb[0:tsz, 128 * j:128 * j + 128],
                                            qT[0:tsz, j, :], alQ[0:tsz, j:j + 1],
                                            None, OP.mult)
                for j, off, tsz in _q_tiles():
                    qb = qtr.tile([128, 128], BF16, tag="qb")
                    nc.tensor.matmul(qb[:, 0:tsz], qn_sb[0:tsz, 128 * j:128 * j + 128],
                                     id16_s[0:tsz, 0:tsz], is_transpose=True,
                                     start=True, stop=True)
                    nc.scalar.copy(qn_fm[:, off:off + tsz], qb[:, 0:tsz])
                for qoff, qsz in Q_CHUNKS:
                    qh = qpj.tile([128, 512], F32, tag="qh")
                    nc.tensor.matmul(qh[0:128, 0:qsz], wqc_s[:, :],
                                     qn_fm[:, qoff:qoff + qsz], start=True, stop=True)
                    if has_bq:
                        nc.scalar.activation(qhT[:, qoff:qoff + qsz], qh[0:128, 0:qsz],
                                             AF.Identity, bias=bq_s[:, 0:1])
                    else:
                        nc.scalar.copy(qhT[:, qoff:qoff + qsz], qh[0:128, 0:qsz])

            # ------------- K + V: centered projections + LN scale columns ----
            # interleaved so PE (proj/transpose), DVE (stats) and ACT (copies)
            # overlap instead of running as three serial phases.
            with contextlib.ExitStack() as kctx:
                kpj = kctx.enter_context(tc.tile_pool(name="k_pj", bufs=2, space="PSUM"))
                ktr = kctx.enter_context(tc.tile_pool(name="k_tr", bufs=2, space="PSUM"))
                vtr = kctx.enter_context(tc.tile_pool(name="v_tr", bufs=2, space="PSUM"))
                vpj = kctx.enter_context(tc.tile_pool(name="v_pj", bufs=2, space="PSUM"))
                kwp = kctx.enter_context(tc.tile_pool(name="k_wp", bufs=3))

                for ci, (coff, csz) in enumerate(K_CHUNKS):
                    pp = kpj.tile([128, KC], F32, tag="pp")
                    nc.tensor.matmul(pp[0:128, 0:csz], wkc_s[:, :],
                                     xk_sb[:, coff:coff + csz], start=True, stop=True)
                    nc.scalar.copy(khT[:, coff:coff + csz], pp[0:128, 0:csz])

                for g in range(0, N_KT, 4):
                    gn = min(4, N_KT - g)
                    tpk = ktr.tile([128, 4, 128], BF16, tag="tpk")
                    tpv = vtr.tile([128, 4, 128], BF16, tag="tpv")
                    for t in range(gn):
                        j = g + t
                        off = j * KT
                        tsz = min(KT, NK - off)
                        nc.tensor.matmul(tpk[0:tsz, t, :], xk_sb[:, off:off + tsz],
                                         id16_s[:, :], is_transpose=True,
                                         start=True, stop=True)
                        nc.tensor.matmul(tpv[0:tsz, t, :], xv_sb[:, off:off + tsz],
                                         id16_s[:, :], is_transpose=True,
                                         start=True, stop=True)
                    for t in range(gn):
                        j = g + t
                        tsz = min(KT, NK - j * KT)
                        nc.vector.bn_stats(b6K[0:tsz, j, :], tpk[0:tsz, t, :])
                        nc.vector.bn_stats(b6V[0:tsz, j, :], tpv[0:tsz, t, :])
                # variance from the even/odd partial stats, vectorized over
                # all ktiles: var = (cv_e + cv_o)/128 + (m_e - m_o)^2/4
                for b6, var in ((b6K, varK), (b6V, varV)):
                    me = b6[:, :, 1:2].rearrange("p t s -> p (t s)")
                    mo = b6[:, :, 4:5].rearrange("p t s -> p (t s)")
                    cve = b6[:, :, 2:3].rearrange("p t s -> p (t s)")
                    cvo = b6[:, :, 5:6].rearrange("p t s -> p (t s)")
                    dmu = kwp.tile([128, N_KT], F32, tag="dmu")
                    nc.vector.tensor_sub(dmu[:, :], me, mo)
                    nc.vector.tensor_scalar(dmu[:, :], dmu[:, :], 0.5, None, OP.mult)
                    dq = kwp.tile([128, N_KT], F32, tag="dq")
                    nc.vector.tensor_mul(dq[:, :], dmu[:, :], dmu[:, :])
                    cvs = kwp.tile([128, N_KT], F32, tag="cvs")
                    nc.vector.tensor_add(cvs[:, :], cve, cvo)
                    nc.vector.scalar_tensor_tensor(var[:, :], cvs[:, :], 1.0 / 128.0,
                                                   dq[:, :], OP.mult, OP.add)
                _var_alpha(nc, kwp, varK, N_KT, alK, eps_s[:, 0:1], al184=al184K)
                _var_alpha(nc, kwp, varV, N_KT, alV, eps_s[:, 0:1])
                # alpha_v folded into the exp instead of the vpack copy:
                # p' = alpha_v * exp(alpha_k * s) via per-partition exp bias;
                # Z then contracts p' against 1/alpha_v to recover sum(p).
                nc.scalar.activation(lnalv[:, :], alV[:, :], AF.Ln)
                nc.vector.tensor_scalar(b184[:, :], lnalv[:, :], EXP_C, EXP_B,
                                        OP.mult, OP.add)
                with nc.allow_low_precision(reason="1/alpha_v Z stationary"):
                    nc.vector.reciprocal(invav[:, :], alV[:, :])

                for g in range(0, N_KT, 4):
                    gn = min(4, N_KT - g)
                    vp = vpj.tile([128, 4, 128], F32, tag="vp")
                    for t in range(gn):
                        j = g + t
                        off = j * KT
                        tsz = min(KT, NK - off)
                        nc.tensor.matmul(vp[0:tsz, t, :], xv_sb[:, off:off + tsz],
                                         wvc_s[:, :], start=True, stop=True)
                    dvv = vpack[:, 132 * g:132 * (g + gn)].rearrange(
                        "p (t h c) -> p t h c", h=HEADS, c=33)
                    nc.scalar.copy(
                        dvv[:, 0:gn, :, 0:32],
                        vp[:, 0:gn, :].rearrange("p t (h c) -> p t h c", c=32))
                # 1/alpha_v into column 32 of every head block
                vz = vpack[:, :].rearrange("p (t h c) -> p t h c", h=HEADS, c=33)
                for h in range(HEADS):
                    nc.vector.tensor_copy(
                        vz[:, :, h, 32:33].rearrange("p t o -> p (t o)"),
                        invav[:, :])

            # ---------------- attention ----------------
            with contextlib.ExitStack() as actx:
                scp = actx.enter_context(tc.tile_pool(name="sc_ps", bufs=1, space="PSUM"))
                avp = actx.enter_context(tc.tile_pool(name="av_ps", bufs=1, space="PSUM"))
                pep = actx.enter_context(tc.tile_pool(name="pexp", bufs=3))
                zwp = actx.enter_context(tc.tile_pool(name="zw", bufs=2))

                for (qoff, qsz) in Q_CHUNKS:
                    avA = avp.tile([128, 512], F32, tag="avA")
                    avB = avp.tile([128, 512], F32, tag="avB")
                    pes = {}

                    def issue_av(jj):
                        koff = jj * KT
                        ksz = min(KT, NK - koff)
                        pea, pe2, pe3 = pes.pop(jj)
                        first = (jj == 0)
                        last = (jj == N_KT - 1)
                        for h in range(HEADS):
                            mv = (pea[0:ksz, h, 0:qsz] if h < 2
                                  else (pe2 if h == 2 else pe3)[0:ksz, 0:qsz])
                            av = avA if h < 2 else avB
                            rbase = 64 * (h % 2)
                            nc.tensor.matmul(
                                av[rbase:rbase + 33, 0:qsz],
                                vpack[0:ksz,
                                      132 * jj + 33 * h:132 * jj + 33 * h + 33],
                                mv, start=first, stop=last,
                                tile_position=(0, rbase),
                                skip_group_check=True)

                    for j, koff, ksz in _k_tiles():
                        sca = scp.tile([128, 2, 512], F32, tag="sca", bufs=2)
                        sc2 = scp.tile([128, 512], F32, tag="sc2", bufs=1)
                        sc3 = scp.tile([128, 512], F32, tag="sc3", bufs=1)
                        for h in range(2):
                            nc.tensor.matmul(
                                sca[0:ksz, h, 0:qsz],
                                khT[32 * h:32 * h + 32, koff:koff + ksz],
                                qhT[32 * h:32 * h + 32, qoff:qoff + qsz],
                                start=True, stop=True, tile_position=(32 * h, 0))
                        nc.tensor.matmul(
                            sc2[0:ksz, 0:qsz],
                            khT[64:96, koff:koff + ksz],
                            qhT[64:96, qoff:qoff + qsz],
                            start=True, stop=True, tile_position=(64, 0))
                        nc.tensor.matmul(
                            sc3[0:ksz, 0:qsz],
                            khT[96:128, koff:koff + ksz],
                            qhT[96:128, qoff:qoff + qsz],
                            start=True, stop=True, tile_position=(96, 0))

                        pea = pep.tile([128, 2, 512], BF16, tag="pea")
                        pe2 = pep.tile([128, 512], BF16, tag="pe2")
                        pe3 = pep.tile([128, 512], BF16, tag="pe3")
                        nc.scalar.activation(pea[0:ksz, :, 0:qsz],
                                             sca[0:ksz, :, 0:qsz], AF.Exp,
                                             bias=lnalv[0:ksz, j:j + 1],
                                             scale=alK[0:ksz, j:j + 1])
                        nc.vector.tensor_scalar(
                            pe2[0:ksz, 0:qsz].bitcast(I16),
                            sc2[0:ksz, 0:qsz],
                            al184K[0:ksz, j:j + 1], b184[0:ksz, j:j + 1],
                            OP.mult, OP.add)
                        nc.vector.tensor_scalar(
                            pe3[0:ksz, 0:qsz].bitcast(I16),
                            sc3[0:ksz, 0:qsz],
                            al184K[0:ksz, j:j + 1], b184[0:ksz, j:j + 1],
                            OP.mult, OP.add)
                        pes[j] = (pea, pe2, pe3)

                        if j >= AV_DELAY:
                            issue_av(j - AV_DELAY)
                    for jj in range(N_KT - AV_DELAY, N_KT):
                        issue_av(jj)

                    # epilogue: Z rows live at rows 32/96 of each av bank.
                    # Gather into a 1.0-filled tile (finite reciprocal), one
                    # recip, two head-map broadcast matmuls, then normalize.
                    z4 = zwp.tile([128, 512], F32, tag="z4")
                    nc.vector.memset(z4[:, 0:qsz], 1.0)
                    nc.vector.tensor_copy(z4[0:1, 0:qsz], avA[32:33, 0:qsz])
                    nc.vector.tensor_copy(z4[32:33, 0:qsz], avA[96:97, 0:qsz])
                    nc.vector.tensor_copy(z4[64:65, 0:qsz], avB[32:33, 0:qsz])
                    nc.vector.tensor_copy(z4[96:97, 0:qsz], avB[96:97, 0:qsz])
                    z4r = zwp.tile([128, 512], BF16, tag="z4r")
                    with nc.allow_low_precision(reason="1/Z softmax scale"):
                        nc.vector.reciprocal(z4r[0:128, 0:qsz], z4[0:128, 0:qsz])
                    zrb = scp.tile([128, 2, 512], F32, tag="sca", bufs=2)
                    nc.tensor.matmul(zrb[0:128, 0, 0:qsz], zhmA_s[:, :],
                                     z4r[0:128, 0:qsz], start=True, stop=True)
                    nc.tensor.matmul(zrb[0:128, 1, 0:qsz], zhmB_s[:, :],
                                     z4r[0:128, 0:qsz], start=True, stop=True)
                    zbcA = zwp.tile([128, 512], BF16, tag="zbcA")
                    zbcB = zwp.tile([128, 512], BF16, tag="zbcB")
                    nc.vector.tensor_copy(zbcA[0:97, 0:qsz], zrb[0:97, 0, 0:qsz])
                    nc.vector.tensor_copy(zbcB[0:97, 0:qsz], zrb[0:97, 1, 0:qsz])
                    for av, aT, zbc in ((avA, aTA, zbcA), (avB, aTB, zbcB)):
                        nc.vector.tensor_mul(aT[0:33, qoff:qoff + qsz],
                                             av[0:33, 0:qsz], zbc[0:33, 0:qsz])
                        nc.vector.tensor_mul(aT[64:97, qoff:qoff + qsz],
                                             av[64:97, 0:qsz], zbc[64:97, 0:qsz])

            # ---------------- back half ----------------
            with contextlib.ExitStack() as bctx:
                zp = bctx.enter_context(tc.tile_pool(name="z_ps", bufs=2, space="PSUM"))
                tp = bctx.enter_context(tc.tile_pool(name="t_ps", bufs=1, space="PSUM"))
                hp = bctx.enter_context(tc.tile_pool(name="h_ps", bufs=1, space="PSUM"))
                bwp = bctx.enter_context(tc.tile_pool(name="bk_work", bufs=3))
                bst = bctx.enter_context(tc.tile_pool(name="bk_stats", bufs=1))

                mv1 = bst.tile([128, 2 * N_QT], F32, tag="mv1")
                mv2 = bst.tile([128, 2 * N_QT], F32, tag="mv2")
                nmu1 = bst.tile([128, N_QT], F32, tag="nmu1")
                rs1 = bst.tile([128, N_QT], F32, tag="rs1")
                nmu2 = bst.tile([128, N_QT], F32, tag="nmu2")
                rs2 = bst.tile([128, N_QT], F32, tag="rs2")

                # proj + skip + pre-LN stats
                for j, off, csz in _q_tiles():
                    zps = zp.tile([128, 128], F32, tag="zps")
                    nc.tensor.matmul(zps[0:csz, :], aTA[:, off:off + csz], pjA_s[:, :],
                                     start=True, stop=False, skip_group_check=True)
                    nc.tensor.matmul(zps[0:csz, :], aTB[:, off:off + csz], pjB_s[:, :],
                                     start=False, stop=True, skip_group_check=True)
                    nc.vector.tensor_add(z_sb[0:csz, 128 * j:128 * j + 128],
                                         zps[0:csz, :],
                                         skip_sb[0:csz, 128 * j:128 * j + 128])
                    bns = bwp.tile([128, 6], F32, tag="bns")
                    nc.vector.bn_stats(bns[0:csz, :], z_sb[0:csz, 128 * j:128 * j + 128])
                    nc.vector.bn_aggr(mv1[0:csz, 2 * j:2 * j + 2], bns[0:csz, :])
                _ln_alpha(nc, bwp, mv1, N_QT, rs1, eps_s[:, 0:1], nmu=nmu1)

                # MLP per chunk + post-LN stats
                for j, off, csz in _q_tiles():
                    zln = bwp.tile([128, 128], BF16, tag="zln")
                    nc.vector.tensor_scalar(zln[0:csz, :], z_sb[0:csz, 128 * j:128 * j + 128],
                                            nmu1[0:csz, j:j + 1], rs1[0:csz, j:j + 1],
                                            OP.add, OP.mult)
                    trz = tp.tile([128, 128], BF16, tag="trz")
                    nc.tensor.matmul(trz[:, 0:csz], zln[0:csz, :], id16_s[0:csz, 0:csz],
                                     is_transpose=True, start=True, stop=True)
                    zlnT = bwp.tile([128, 128], BF16, tag="zlnT")
                    nc.vector.tensor_copy(zlnT[:, 0:csz], trz[:, 0:csz])
                    hg = bwp.tile([128, 2, 128], BF16, tag="hg")
                    for bi, w1s in ((0, w1a_s), (1, w1b_s)):
                        hps = hp.tile([128, 128], F32, tag=f"hps{bi}")
                        nc.tensor.matmul(hps[0:128, 0:csz], w1s[:, :], zlnT[:, 0:csz],
                                         start=True, stop=True)
                        gb = (b1a_s if bi == 0 else b1b_s)
                        nc.scalar.activation(hg[:, bi, 0:csz], hps[0:128, 0:csz],
                                             AF.Gelu,
                                             bias=(gb[:, 0:1] if has_b1 else 0.0))
                    mps = zp.tile([128, 128], F32, tag="mps")
                    nc.tensor.matmul(mps[0:csz, :], hg[:, 0, 0:csz], w2a_s[:, :],
                                     start=True, stop=False, skip_group_check=True)
                    nc.tensor.matmul(mps[0:csz, :], hg[:, 1, 0:csz], w2b_s[:, :],
                                     start=False, stop=not has_b2,
                                     skip_group_check=True)
                    if has_b2:
                        nc.tensor.matmul(mps[0:csz, :], ones16_s[0:1, 0:csz],
                                         b2_s[0:1, :], start=False, stop=True,
                                         skip_group_check=True)
                    zr2 = bwp.tile([128, 128], F32, tag="zr2")
                    nc.vector.tensor_add(zr2[0:csz, :], mps[0:csz, :],
                                         z_sb[0:csz, 128 * j:128 * j + 128])
                    nc.vector.tensor_copy(z_sb[0:csz, 128 * j:128 * j + 128], zr2[0:csz, :])
                    bns2 = bwp.tile([128, 6], F32, tag="bns2")
                    nc.vector.bn_stats(bns2[0:csz, :], zr2[0:csz, :])
                    nc.vector.bn_aggr(mv2[0:csz, 2 * j:2 * j + 2], bns2[0:csz, :])
                _ln_alpha(nc, bwp, mv2, N_QT, rs2, eps_s[:, 0:1], nmu=nmu2)

                for j, off, csz in _q_tiles():
                    zo = bwp.tile([128, 128], F32, tag="zo")
                    nc.vector.tensor_scalar(zo[0:csz, :], z_sb[0:csz, 128 * j:128 * j + 128],
                                            nmu2[0:csz, j:j + 1], rs2[0:csz, j:j + 1],
                                            OP.add, OP.mult)
                    if has_post:
                        zo2 = bwp.tile([128, 128], F32, tag="zo2")
                        nc.vector.tensor_mul(zo2[0:csz, :], zo[0:csz, :],
                                             pog_s[0:csz, :])
                        nc.vector.tensor_add(zo[0:csz, :], zo2[0:csz, :],
                                             pob_s[0:csz, :])
                    tro = tp.tile([128, 128], F32, tag="tro")
                    nc.tensor.matmul(tro[:, 0:csz], zo[0:csz, :], id32_s[0:csz, 0:csz],
                                     is_transpose=True, start=True, stop=True)
                    nc.vector.tensor_copy(outfm[:, off:off + csz], tro[:, 0:csz])

                nc.sync.dma_start(out=y[:], in_=outfm[:])

    return nc


def _host_prep(inputs):
    f = np.float32
    bf = ml_dtypes.bfloat16
    g = {}
    scale = np.float32(DHEAD ** -0.5)
    wq_e = (np.asarray(inputs["ln_q_g"], f)[:, None] * np.asarray(inputs["wq"], f)) * scale
    bq_e = (np.asarray(inputs["ln_q_b"], f) @ np.asarray(inputs["wq"], f)
            + np.asarray(inputs["bq"], f)) * scale
    wk_e = np.asarray(inputs["ln_k_g"], f)[:, None] * np.asarray(inputs["wk"], f)
    wv_e = np.asarray(inputs["ln_v_g"], f)[:, None] * np.asarray(inputs["wv"], f)
    bv_e = (np.asarray(inputs["ln_v_b"], f) @ np.asarray(inputs["wv"], f)
            + np.asarray(inputs["bv"], f))
    # mean-centering folded into weights: (x - mu) @ W == x @ (W - colsum/D)
    wqc = wq_e - wq_e.sum(0, keepdims=True) / D
    wkc = wk_e - wk_e.sum(0, keepdims=True) / D
    wvc = wv_e - wv_e.sum(0, keepdims=True) / D

    proj_w = np.asarray(inputs["proj_w"], f)
    proj_b_eff = np.asarray(inputs["proj_b"], f) + bv_e @ proj_w
    pjA = np.zeros((128, 128), f)
    pjB = np.zeros((128, 128), f)
    pjA[0:32] = proj_w[0:32]
    pjA[64:96] = proj_w[32:64]
    pjB[0:32] = proj_w[64:96]
    pjB[64:96] = proj_w[96:128]

    # broadcast maps: z4r rows {0,32,64,96} hold 1/Z for heads 0..3
    zhmA = np.zeros((128, 128), f)
    zhmB = np.zeros((128, 128), f)
    zhmA[0, 0:33] = 1.0
    zhmA[32, 64:97] = 1.0
    zhmB[64, 0:33] = 1.0
    zhmB[96, 64:97] = 1.0

    pre_g = np.asarray(inputs["pre_g"], f)
    pre_b = np.asarray(inputs["pre_b"], f)
    w1_e = pre_g[:, None] * np.asarray(inputs["mlp_w1"], f)
    b1_e = pre_b @ np.asarray(inputs["mlp_w1"], f) + np.asarray(inputs["mlp_b1"], f)
    w2 = np.asarray(inputs["mlp_w2"], f)
    b2_e = np.asarray(inputs["mlp_b2"], f)
    post_g = np.asarray(inputs["post_g"], f)
    post_b = np.asarray(inputs["post_b"], f)

    g["wqc"] = np.ascontiguousarray(wqc.astype(bf))
    g["wkc"] = np.ascontiguousarray(wkc.astype(bf))
    g["wvc"] = np.ascontiguousarray(wvc.astype(bf))
    g["pjA"] = pjA.astype(bf)
    g["pjB"] = pjB.astype(bf)
    g["zhmA"] = np.ascontiguousarray(zhmA.astype(bf))
    g["zhmB"] = np.ascontiguousarray(zhmB.astype(bf))
    g["w1a"] = np.ascontiguousarray(w1_e[:, 0:128].astype(bf))
    g["w1b"] = np.ascontiguousarray(w1_e[:, 128:256].astype(bf))
    g["w2a"] = np.ascontiguousarray(w2[0:128].astype(bf))
    g["w2b"] = np.ascontiguousarray(w2[128:256].astype(bf))
    g["id16"] = np.eye(128, dtype=bf)
    g["id32"] = np.eye(128, dtype=f)
    g["ones16"] = np.ones((128, 128), bf)

    flags = {
        "has_bq": bool(np.any(bq_e != 0)),
        "has_b1": bool(np.any(b1_e != 0)),
        "has_b2": bool(np.any(b2_e != 0)),
        "has_post": not (np.allclose(post_g, 1.0) and np.allclose(post_b, 0.0)),
    }
    if flags["has_bq"]:
        g["bqcol"] = np.ascontiguousarray(bq_e[:, None], dtype=f)
    if flags["has_b1"]:
        g["b1acol"] = np.ascontiguousarray(b1_e[0:128, None], dtype=f)
        g["b1bcol"] = np.ascontiguousarray(b1_e[128:256, None], dtype=f)
    if flags["has_b2"]:
        g["b2row"] = np.ascontiguousarray(b2_e[None, :].astype(bf))
    if flags["has_post"]:
        g["pogb"] = np.ascontiguousarray(np.broadcast_to(post_g[None, :], (128, 128)), f)
        g["pobb"] = np.ascontiguousarray(np.broadcast_to(post_b[None, :], (128, 128)), f)
    return g, flags, proj_b_eff


STARTS = [0, 938, 1876, 2813]
LENS = [938, 938, 937, 937]


def _make_in_maps(inputs):
    f = np.float32
    bf = ml_dtypes.bfloat16
    q = np.asarray(inputs["q"], f)
    k = np.asarray(inputs["k"], f)
    v = np.asarray(inputs["v"], f)
    skip = np.asarray(inputs["skip"], f)
    consts, flags, proj_b_eff = _host_prep(inputs)

    in_maps = []
    for c in range(8):
        b, s = c // 4, c % 4
        qfm = np.ascontiguousarray(q[b].transpose(1, 0, 2).reshape(128, NQ_FULL))
        sfm = np.ascontiguousarray(skip[b].transpose(1, 0, 2).reshape(128, NQ_FULL))
        kfm = np.ascontiguousarray(k[b].transpose(1, 0, 2, 3).reshape(128, NK))
        vfm = np.ascontiguousarray(v[b].transpose(1, 0, 2, 3).reshape(128, NK))
        xq = np.zeros((128, TQ), bf)
        xq[:, :LENS[s]] = qfm[:, STARTS[s]:STARTS[s] + LENS[s]].astype(bf)
        sk = np.zeros((128, TQ), f)
        sk[:, :LENS[s]] = sfm[:, STARTS[s]:STARTS[s] + LENS[s]]
        # token-major skip tiles with proj bias folded in
        skip_tm = np.zeros((128, N_QT * 128), f)
        for j in range(N_QT):
            off = j * KT
            tsz = min(KT, TQ - off)
            skip_tm[0:tsz, 128 * j:128 * j + 128] = sk[:, off:off + tsz].T + proj_b_eff[None, :]
        m = {"xq": xq, "xk": kfm.astype(bf), "xv": vfm.astype(bf),
             "xskip": skip_tm}
        m.update(consts)
        in_maps.append(m)
    return in_maps, flags


_CACHE = {}


def kernel(**inputs):
    f = np.float32
    in_maps, flags = _make_in_maps(inputs)

    key = tuple(sorted(flags.items()))
    if key not in _CACHE:
        _CACHE[key] = build_program(flags)
    nc = _CACHE[key]

    _install_compile_patch()
    res = bass_utils.run_bass_kernel_spmd(nc, in_maps, core_ids=list(range(8)))

    full = np.zeros((B, 128, NQ_FULL), f)
    for c in range(8):
        b, s = c // 4, c % 4
        full[b][:, STARTS[s]:STARTS[s] + LENS[s]] = res.results[c]["y"][:, :LENS[s]]
    return np.ascontiguousarray(
        full.reshape(B, 128, N, M).transpose(0, 2, 1, 3))


# revision 48
# speedup vs baseline: 1.0152x; 1.0152x over previous
"""Trainium2 Bass kernel for nn_CrossAttention (B=2,N=6,D=128,M=625,H=28,W=60, 4 heads x 32).

Sharding: 8 cores = 2 batches x 4 query-token shards. Zero collectives.

v2 restructure (from the 590us baseline):
- attention j-loop de-serialized: the baseline's critical cycle was
  EXP(j) -> AV(j) -> score(j+1) -> EXP(j+1) at 2.65us/iter because PSUM
  had no room to double-buffer score tiles. New PSUM map (8 banks exact):
  sca (heads 0,1; 2 banks x 2 bufs) + sc2 + sc3 (1 bank, single-buffered,
  fine-grained DVE chains) + av (1 bank: 4 heads x 32 col-tiled, no Z
  column) + zden (1 bank: softmax denominators via ones-stationary
  matmuls at partitions 0/32/64/96).
- AV/Z matmuls delayed by 2 ktiles behind the scores so the PE never
  stalls on the activations; probabilities triple-buffered in SBUF.
- softmax normalize: strided-partition gather of the 4 Z rows -> one
  reciprocal -> one PE broadcast matmul (head-map stationary) -> one
  tensor_mul. Replaces the 26us of 1-partition reciprocals.
- proj is one 128x128 matmul per q-tile (head-major aT layout).
Everything else (LN folds, bf16 operands, split exact/bit-trick exp)
as in the baseline.
"""

import numpy as np
import ml_dtypes

import concourse.bass as bass
import concourse.mybir as mybir
import concourse.tile as tile
from concourse import bass_utils
from concourse.vector_clock import ScopedClock, VectorClock
from concourse.tile_scheduler import N_PROCS

F32 = mybir.dt.float32
BF16 = mybir.dt.bfloat16
I16 = mybir.dt.int16
AF = mybir.ActivationFunctionType
OP = mybir.AluOpType

B, N, D, M, H, W = 2, 6, 128, 625, 28, 60
HEADS, DHEAD = 4, 32
NQ_FULL = N * M            # 3750
NK = N * H * W             # 10080
TQ = 938                   # padded per-core query shard
EPS = 1e-5
EXP_C = 184.66496          # 128 * log2(e): bf16-bits exp trick multiplier
EXP_B = 16256.0            # 127 * 128: bf16 exponent bias in bit space

KT = 128
N_KT = (NK + KT - 1) // KT          # 79 (last = 96)
N_QT = (TQ + KT - 1) // KT          # 8  (last = 42)
Q_CHUNKS = [(0, 512), (512, TQ - 512)]
KC = 512
K_CHUNKS = [(o, min(KC, NK - o)) for o in range(0, NK, KC)]   # 20 (last = 352)
AV_DELAY = 2


def _k_tiles():
    for j in range(N_KT):
        off = j * KT
        yield j, off, min(KT, NK - off)


def _q_tiles():
    for j in range(N_QT):
        off = j * KT
        yield j, off, min(KT, TQ - off)


def _split_multiwait_json(bir_json: bytes) -> bytes:
    """This walrus build allows only one sync-wait per instruction: move
    extra on_wait entries onto EventSemaphore instructions inserted just
    before the owner (same engine, so ordering is preserved)."""
    import json
    bir = json.loads(bir_json)
    for fn in bir["functions"]:
        for blk in fn["blocks"]:
            out = []
            for ins in blk["instructions"]:
                si = ins.get("sync_info")
                waits = (si or {}).get("on_wait") or []
                if len(waits) > 1:
                    for wi, w in enumerate(waits[:-1]):
                        out.append({
                            "debug": ins.get("debug", 0),
                            "engine": ins["engine"],
                            "ins": [], "outs": [],
                            "name": f"{ins['name']}-xw{wi}",
                            "opcode": "EventSemaphore",
                            "sync_info": {"on_update": [], "on_wait": [w]},
                        })
                    si["on_wait"] = [waits[-1]]
                out.append(ins)
            blk["instructions"] = out
    return json.dumps(bir).encode()


def _install_compile_patch():
    from concourse import bass_utils as bu
    if getattr(bu, "_mw_patched", False):
        return
    orig = bu.compile_bir_kernel

    def patched(bir_json, tmpdir, neff_name="file.neff"):
        return orig(_split_multiwait_json(bir_json), tmpdir, neff_name)

    bu.compile_bir_kernel = patched
    bu._mw_patched = True
    try:
        from concourse import bass2jax
        if getattr(bass2jax, "compile_bir_kernel", None) is orig:
            bass2jax.compile_bir_kernel = patched
    except ImportError:
        pass


class _SplitDrainTileContext(tile.TileContext):
    """This walrus build rejects >1 sem wait on a Drain; split the exit
    drain's waits across per-proc drains (one wait each)."""

    def _drain_and_barrier(self, tick_clock, wait_clock):
        full = tick_clock.global_clock
        for p in range(N_PROCS):
            mask = VectorClock([(1 << 30) if i == p else 0 for i in range(N_PROCS)])
            partial = full.copy()
            partial.elementwise_min(mask)
            d = self.nc.sync.drain()
            wait_clock.add_sem_waits(d.ins, ScopedClock({None: partial}))
        self.nc.all_engine_barrier()
        assert self.sems is not None
        popped = self.nc._tile_sem_poison_stack.pop()
        assert popped is self._sem_poison
        self.nc.clear_and_free_semaphores(list(self.sems.allocated().values()))
        self.nc.all_engine_barrier()


def _var_alpha(nc, wp, var, n_tiles, al, eps_ap, al184=None):
    """al = rsqrt(var+eps) from a compact [128, n] variance tile."""
    sd = wp.tile([128, n_tiles], F32, tag="vasd")
    nc.scalar.activation(sd[:, :], var[:, :], AF.Sqrt, bias=eps_ap)
    nc.vector.reciprocal(al[:, :], sd[:, :])
    if al184 is not None:
        nc.vector.tensor_scalar(al184[:, :], al[:, :], EXP_C, None, OP.mult)


def _ln_alpha(nc, wp, mv, n_tiles, al, eps_ap, al184=None, nmu=None):
    """From interleaved bn_aggr stats mv [128, 2*n]: al = rsqrt(var+eps),
    optionally al184 = al*EXP_C and nmu = -mean."""
    mvv = mv[:, :].rearrange("p (t two) -> p t two", two=2)
    var_ap = mvv[:, :, 1:2].rearrange("p t o -> p (t o)")
    sd = wp.tile([128, n_tiles], F32, tag="lnsd")
    nc.scalar.activation(sd[:, :], var_ap, AF.Sqrt, bias=eps_ap)
    nc.vector.reciprocal(al[:, :], sd[:, :])
    if al184 is not None:
        nc.vector.tensor_scalar(al184[:, :], al[:, :], EXP_C, None, OP.mult)
    if nmu is not None:
        nc.vector.tensor_scalar(
            nmu[:, :], mvv[:, :, 0:1].rearrange("p t o -> p (t o)"),
            -1.0, None, OP.mult)


def build_program(host):
    nc = bass.Bass()

    def inp(name, shape, dt=BF16):
        return nc.dram_tensor(name, list(shape), dt, kind="ExternalInput")

    xq = inp("xq", (128, TQ))
    xk = inp("xk", (128, NK))
    xv = inp("xv", (128, NK))
    xskip = inp("xskip", (128, N_QT * 128), F32)
    wqc = inp("wqc", (128, 128))
    wkc = inp("wkc", (128, 128))
    wvc = inp("wvc", (128, 128))
    pjW = inp("pjW", (128, 128))
    zhmA = inp("zhmA", (128, 128))
    w1a = inp("w1a", (128, 128))
    w1b = inp("w1b", (128, 128))
    w2a = inp("w2a", (128, 128))
    w2b = inp("w2b", (128, 128))
    id16 = inp("id16", (128, 128))
    id32 = inp("id32", (128, 128), F32)
    ones16 = inp("ones16", (128, 128))
    y = nc.dram_tensor("y", [128, TQ], F32, kind="ExternalOutput")

    has_bq = host["has_bq"]
    has_b1 = host["has_b1"]
    has_b2 = host["has_b2"]
    has_post = host["has_post"]
    if has_bq:
        bqcol = inp("bqcol", (128, 1), F32)
    if has_b1:
        b1acol = inp("b1acol", (128, 1), F32)
        b1bcol = inp("b1bcol", (128, 1), F32)
    if has_b2:
        b2row = inp("b2row", (1, 128))
    if has_post:
        pogb = inp("pogb", (128, 128), F32)
        pobb = inp("pobb", (128, 128), F32)

    with _SplitDrainTileContext(nc) as tc:
        import contextlib
        with contextlib.ExitStack() as ctx:
            cpool = ctx.enter_context(tc.tile_pool(name="consts", bufs=1))
            big = ctx.enter_context(tc.tile_pool(name="big", bufs=1))

            def load_const(t, shape, dt=BF16):
                s = cpool.tile(list(shape), dt, tag=t.name)
                nc.sync.dma_start(out=s[:], in_=t[:])
                return s

            wqc_s = load_const(wqc, (128, 128))
            wkc_s = load_const(wkc, (128, 128))
            wvc_s = load_const(wvc, (128, 128))
            pjW_s = load_const(pjW, (128, 128))
            zhmA_s = load_const(zhmA, (128, 128))
            w1a_s = load_const(w1a, (128, 128))
            w1b_s = load_const(w1b, (128, 128))
            w2a_s = load_const(w2a, (128, 128))
            w2b_s = load_const(w2b, (128, 128))
            id16_s = load_const(id16, (128, 128))
            id32_s = load_const(id32, (128, 128), F32)
            ones16_s = load_const(ones16, (128, 128))
            bq_s = load_const(bqcol, (128, 1), F32) if has_bq else None
            b1a_s = load_const(b1acol, (128, 1), F32) if has_b1 else None
            b1b_s = load_const(b1bcol, (128, 1), F32) if has_b1 else None
            b2_s = load_const(b2row, (1, 128)) if has_b2 else None
            if has_post:
                pog_s = load_const(pogb, (128, 128), F32)
                pob_s = load_const(pobb, (128, 128), F32)

            eps_s = cpool.tile([128, 1], F32, tag="eps")
            nc.vector.memset(eps_s[:, :], EPS)

            xq_sb = big.tile([128, TQ], BF16, tag="xq_sb")
            xk_sb = big.tile([128, NK], BF16, tag="xk_sb")
            xv_sb = big.tile([128, NK], BF16, tag="xv_sb")
            skip_sb = big.tile([128, N_QT * 128], F32, tag="skip_sb")
            khT = big.tile([128, NK], BF16, tag="khT")
            qhT = big.tile([128, TQ], BF16, tag="qhT")
            vpack = big.tile([128, N_KT * 128], BF16, tag="vpack")
            aT = big.tile([128, TQ], BF16, tag="aT")
            z_sb = big.tile([128, N_QT * 128], F32, tag="z_sb")
            outfm = big.tile([128, TQ], F32, tag="outfm")
            qn_sb = big.tile([128, N_QT * 128], BF16, tag="qn_sb")
            qn_fm = big.tile([128, TQ], BF16, tag="qn_fm")
            alK = big.tile([128, N_KT], F32, tag="alK")
            al184K = big.tile([128, N_KT], F32, tag="al184K")
            alV = big.tile([128, N_KT], F32, tag="alV")
            lnalv = big.tile([128, N_KT], F32, tag="lnalv")
            b184 = big.tile([128, N_KT], F32, tag="b184")
            invav = big.tile([128, N_KT], BF16, tag="invav")
            b6K = big.tile([128, N_KT, 6], F32, tag="b6K")
            b6V = big.tile([128, N_KT, 6], F32, tag="b6V")
            varK = big.tile([128, N_KT], F32, tag="varK")
            varV = big.tile([128, N_KT], F32, tag="varV")
            alQ = big.tile([128, N_QT], F32, tag="alQ")
            mvQ = big.tile([128, 2 * N_QT], F32, tag="mvQ")

            nc.sync.dma_start(out=xq_sb[:], in_=xq[:])
            # chunked so the first transposes start after ~1/4 of the load
            for do in range(0, NK, 2560):
                dn = min(2560, NK - do)
                nc.sync.dma_start(out=xk_sb[:, do:do + dn], in_=xk[:, do:do + dn])
                nc.sync.dma_start(out=xv_sb[:, do:do + dn], in_=xv[:, do:do + dn])
            nc.sync.dma_start(out=skip_sb[:], in_=xskip[:])

            # ---------------- Q: full LN (token-major) + projection ----------
            with contextlib.ExitStack() as qctx:
                qps = qctx.enter_context(tc.tile_pool(name="q_ps", bufs=1, space="PSUM"))
                qtr = qctx.enter_context(tc.tile_pool(name="q_tr", bufs=2, space="PSUM"))
                qpj = qctx.enter_context(tc.tile_pool(name="q_pj", bufs=2, space="PSUM"))
                qwp = qctx.enter_context(tc.tile_pool(name="q_wp", bufs=3))

                qT = qps.tile([128, N_QT, 128], BF16, tag="qT")
                for j, off, tsz in _q_tiles():
                    nc.tensor.matmul(qT[0:tsz, j, :], xq_sb[:, off:off + tsz],
                                     id16_s[:, :], is_transpose=True,
                                     start=True, stop=True)
                for j, off, tsz in _q_tiles():
                    b6 = qwp.tile([128, 6], F32, tag="qb6")
                    nc.vector.bn_stats(b6[0:tsz, :], qT[0:tsz, j, :])
                    nc.vector.bn_aggr(mvQ[0:tsz, 2 * j:2 * j + 2], b6[0:tsz, :])
                _ln_alpha(nc, qwp, mvQ, N_QT, alQ, eps_s[:, 0:1])
                for j, off, tsz in _q_tiles():
                    nc.vector.tensor_scalar(qn_sb[0:tsz, 128 * j:128 * j + 128],
                                            qT[0:tsz, j, :], alQ[0:tsz, j:j + 1],
                                            None, OP.mult)
                for j, off, tsz in _q_tiles():
                    qb = qtr.tile([128, 128], BF16, tag="qb")
                    nc.tensor.matmul(qb[:, 0:tsz], qn_sb[0:tsz, 128 * j:128 * j + 128],
                                     id16_s[0:tsz, 0:tsz], is_transpose=True,
                                     start=True, stop=True)
                    nc.scalar.copy(qn_fm[:, off:off + tsz], qb[:, 0:tsz])
                for qoff, qsz in Q_CHUNKS:
                    qh = qpj.tile([128, 512], F32, tag="qh")
                    nc.tensor.matmul(qh[0:128, 0:qsz], wqc_s[:, :],
                                     qn_fm[:, qoff:qoff + qsz], start=True, stop=True)
                    if has_bq:
                        nc.scalar.activation(qhT[:, qoff:qoff + qsz], qh[0:128, 0:qsz],
                                             AF.Identity, bias=bq_s[:, 0:1])
                    else:
                        nc.scalar.copy(qhT[:, qoff:qoff + qsz], qh[0:128, 0:qsz])

            # ------------- K + V: centered projections + LN scale columns ----
            # interleaved so PE (proj/transpose), DVE (stats) and ACT (copies)
            # overlap instead of running as three serial phases.
            with contextlib.ExitStack() as kctx:
                kpj = kctx.enter_context(tc.tile_pool(name="k_pj", bufs=2, space="PSUM"))
                ktr = kctx.enter_context(tc.tile_pool(name="k_tr", bufs=2, space="PSUM"))
                vtr = kctx.enter_context(tc.tile_pool(name="v_tr", bufs=2, space="PSUM"))
                vpj = kctx.enter_context(tc.tile_pool(name="v_pj", bufs=2, space="PSUM"))
                kwp = kctx.enter_context(tc.tile_pool(name="k_wp", bufs=3))

                for ci, (coff, csz) in enumerate(K_CHUNKS):
                    pp = kpj.tile([128, KC], F32, tag="pp")
                    nc.tensor.matmul(pp[0:128, 0:csz], wkc_s[:, :],
                                     xk_sb[:, coff:coff + csz], start=True, stop=True)
                    nc.scalar.copy(khT[:, coff:coff + csz], pp[0:128, 0:csz])

                for g in range(0, N_KT, 4):
                    gn = min(4, N_KT - g)
                    tpk = ktr.tile([128, 4, 128], BF16, tag="tpk")
                    tpv = vtr.tile([128, 4, 128], BF16, tag="tpv")
                    for t in range(gn):
                        j = g + t
                        off = j * KT
                        tsz = min(KT, NK - off)
                        nc.tensor.matmul(tpk[0:tsz, t, :], xk_sb[:, off:off + tsz],
                                         id16_s[:, :], is_transpose=True,
                                         start=True, stop=True)
                        nc.tensor.matmul(tpv[0:tsz, t, :], xv_sb[:, off:off + tsz],
                                         id16_s[:, :], is_transpose=True,
                                         start=True, stop=True)
                    for t in range(gn):
                        j = g + t
                        tsz = min(KT, NK - j * KT)
                        nc.vector.bn_stats(b6K[0:tsz, j, :], tpk[0:tsz, t, :])
                        nc.vector.bn_stats(b6V[0:tsz, j, :], tpv[0:tsz, t, :])
                # variance from the even/odd partial stats, vectorized over
                # all ktiles: var = (cv_e + cv_o)/128 + (m_e - m_o)^2/4
                for b6, var in ((b6K, varK), (b6V, varV)):
                    me = b6[:, :, 1:2].rearrange("p t s -> p (t s)")
                    mo = b6[:, :, 4:5].rearrange("p t s -> p (t s)")
                    cve = b6[:, :, 2:3].rearrange("p t s -> p (t s)")
                    cvo = b6[:, :, 5:6].rearrange("p t s -> p (t s)")
                    dmu = kwp.tile([128, N_KT], F32, tag="dmu")
                    nc.vector.tensor_sub(dmu[:, :], me, mo)
                    nc.vector.tensor_scalar(dmu[:, :], dmu[:, :], 0.5, None, OP.mult)
                    dq = kwp.tile([128, N_KT], F32, tag="dq")
                    nc.vector.tensor_mul(dq[:, :], dmu[:, :], dmu[:, :])
                    cvs = kwp.tile([128, N_KT], F32, tag="cvs")
                    nc.vector.tensor_add(cvs[:, :], cve, cvo)
                    nc.vector.scalar_tensor_tensor(var[:, :], cvs[:, :], 1.0 / 128.0,
                                                   dq[:, :], OP.mult, OP.add)
                _var_alpha(nc, kwp, varK, N_KT, alK, eps_s[:, 0:1], al184=al184K)
                _var_alpha(nc, kwp, varV, N_KT, alV, eps_s[:, 0:1])
                # alpha_v folded into the exp instead of the vpack copy:
                # p' = alpha_v * exp(alpha_k * s) via per-partition exp bias;
                # Z then contracts p' against 1/alpha_v to recover sum(p).
                nc.scalar.activation(lnalv[:, :], alV[:, :], AF.Ln)
                nc.vector.tensor_scalar(b184[:, :], lnalv[:, :], EXP_C, EXP_B,
                                        OP.mult, OP.add)
                with nc.allow_low_precision(reason="1/alpha_v Z stationary"):
                    nc.vector.reciprocal(invav[:, :], alV[:, :])

                for g in range(0, N_KT, 4):
                    gn = min(4, N_KT - g)
                    vp = vpj.tile([128, 4, 128], F32, tag="vp")
                    for t in range(gn):
                        j = g + t
                        off = j * KT
                        tsz = min(KT, NK - off)
                        nc.tensor.matmul(vp[0:tsz, t, :], xv_sb[:, off:off + tsz],
                                         wvc_s[:, :], start=True, stop=True)
                    nc.scalar.copy(
                        vpack[:, 128 * g:128 * (g + gn)],
                        vp[:, 0:gn, :].rearrange("p a b -> p (a b)"))

            # ---------------- attention ----------------
            with contextlib.ExitStack() as actx:
                scp = actx.enter_context(tc.tile_pool(name="sc_ps", bufs=1, space="PSUM"))
                avp = actx.enter_context(tc.tile_pool(name="av_ps", bufs=1, space="PSUM"))
                pep = actx.enter_context(tc.tile_pool(name="pexp", bufs=3))
                zwp = actx.enter_context(tc.tile_pool(name="zw", bufs=2))

                for (qoff, qsz) in Q_CHUNKS:
                    av = avp.tile([128, 512], F32, tag="av")
                    zden = avp.tile([128, 512], F32, tag="zden")
                    pes = {}

                    def issue_av(jj):
                        koff = jj * KT
                        ksz = min(KT, NK - koff)
                        pea, pe2, pe3 = pes.pop(jj)
                        first = (jj == 0)
                        last = (jj == N_KT - 1)
                        for h in range(HEADS):
                            mv = (pea[0:ksz, h, 0:qsz] if h < 2
                                  else (pe2 if h == 2 else pe3)[0:ksz, 0:qsz])
                            nc.tensor.matmul(
                                av[32 * h:32 * h + 32, 0:qsz],
                                vpack[0:ksz, 128 * jj + 32 * h:128 * jj + 32 * h + 32],
                                mv, start=first, stop=last,
                                tile_position=(0, 32 * h),
                                skip_group_check=True)
                        for h in range(HEADS):
                            mv = (pea[0:ksz, h, 0:qsz] if h < 2
                                  else (pe2 if h == 2 else pe3)[0:ksz, 0:qsz])
                            nc.tensor.matmul(
                                zden[32 * h:32 * h + 1, 0:qsz],
                                invav[0:ksz, jj:jj + 1],
                                mv, start=first, stop=last,
                                tile_position=(0, 32 * h),
                                skip_group_check=True)

                    for j, koff, ksz in _k_tiles():
                        sca = scp.tile([128, 2, 512], F32, tag="sca", bufs=2)
                        sc2 = scp.tile([128, 512], F32, tag="sc2", bufs=1)
                        sc3 = scp.tile([128, 512], F32, tag="sc3", bufs=1)
                        for h in range(2):
                            nc.tensor.matmul(
                                sca[0:ksz, h, 0:qsz],
                                khT[32 * h:32 * h + 32, koff:koff + ksz],
                                qhT[32 * h:32 * h + 32, qoff:qoff + qsz],
                                start=True, stop=True, tile_position=(32 * h, 0))
                        nc.tensor.matmul(
                            sc2[0:ksz, 0:qsz],
                            khT[64:96, koff:koff + ksz],
                            qhT[64:96, qoff:qoff + qsz],
                            start=True, stop=True, tile_position=(64, 0))
                        nc.tensor.matmul(
                            sc3[0:ksz, 0:qsz],
                            khT[96:128, koff:koff + ksz],
                            qhT[96:128, qoff:qoff + qsz],
                            start=True, stop=True, tile_position=(96, 0))

                        pea = pep.tile([128, 2, 512], BF16, tag="pea")
                        pe2 = pep.tile([128, 512], BF16, tag="pe2")
                        pe3 = pep.tile([128, 512], BF16, tag="pe3")
                        nc.scalar.activation(pea[0:ksz, :, 0:qsz],
                                             sca[0:ksz, :, 0:qsz], AF.Exp,
                                             bias=lnalv[0:ksz, j:j + 1],
                                             scale=alK[0:ksz, j:j + 1])
                        nc.vector.tensor_scalar(
                            pe2[0:ksz, 0:qsz].bitcast(I16),
                            sc2[0:ksz, 0:qsz],
                            al184K[0:ksz, j:j + 1], b184[0:ksz, j:j + 1],
                            OP.mult, OP.add)
                        nc.vector.tensor_scalar(
                            pe3[0:ksz, 0:qsz].bitcast(I16),
                            sc3[0:ksz, 0:qsz],
                            al184K[0:ksz, j:j + 1], b184[0:ksz, j:j + 1],
                            OP.mult, OP.add)
                        pes[j] = (pea, pe2, pe3)

                        if j >= AV_DELAY:
                            issue_av(j - AV_DELAY)
                    for jj in range(N_KT - AV_DELAY, N_KT):
                        issue_av(jj)

                    # epilogue: normalize all 4 heads by Z. Z rows sit at
                    # partitions 0/32/64/96; gather into an SBUF tile whose
                    # other rows are 1.0 so the full-tile reciprocal stays
                    # finite, then head-map matmul broadcasts 1/Z to all rows.
                    z4 = zwp.tile([128, 512], F32, tag="z4")
                    nc.vector.memset(z4[:, 0:qsz], 1.0)
                    for h in range(HEADS):
                        nc.vector.tensor_copy(z4[32 * h:32 * h + 1, 0:qsz],
                                              zden[32 * h:32 * h + 1, 0:qsz])
                    z4r = zwp.tile([128, 512], BF16, tag="z4r")
                    with nc.allow_low_precision(reason="1/Z softmax scale"):
                        nc.vector.reciprocal(z4r[0:128, 0:qsz], z4[0:128, 0:qsz])
                    zrb = scp.tile([128, 2, 512], F32, tag="sca", bufs=2)
                    nc.tensor.matmul(zrb[0:128, 0, 0:qsz], zhmA_s[:, :],
                                     z4r[0:128, 0:qsz], start=True, stop=True)
                    zbc = zwp.tile([128, 512], BF16, tag="zbc")
                    nc.vector.tensor_copy(zbc[0:128, 0:qsz], zrb[0:128, 0, 0:qsz])
                    nc.vector.tensor_mul(aT[:, qoff:qoff + qsz],
                                         av[0:128, 0:qsz], zbc[0:128, 0:qsz])

            # ---------------- back half ----------------
            with contextlib.ExitStack() as bctx:
                zp = bctx.enter_context(tc.tile_pool(name="z_ps", bufs=2, space="PSUM"))
                tp = bctx.enter_context(tc.tile_pool(name="t_ps", bufs=1, space="PSUM"))
                hp = bctx.enter_context(tc.tile_pool(name="h_ps", bufs=1, space="PSUM"))
                bwp = bctx.enter_context(tc.tile_pool(name="bk_work", bufs=3))
                bst = bctx.enter_context(tc.tile_pool(name="bk_stats", bufs=1))

                mv1 = bst.tile([128, 2 * N_QT], F32, tag="mv1")
                mv2 = bst.tile([128, 2 * N_QT], F32, tag="mv2")
                nmu1 = bst.tile([128, N_QT], F32, tag="nmu1")
                rs1 = bst.tile([128, N_QT], F32, tag="rs1")
                nmu2 = bst.tile([128, N_QT], F32, tag="nmu2")
                rs2 = bst.tile([128, N_QT], F32, tag="rs2")

                # proj + skip + pre-LN stats
                for j, off, csz in _q_tiles():
                    zps = zp.tile([128, 128], F32, tag="zps")
                    nc.tensor.matmul(zps[0:csz, :], aT[:, off:off + csz], pjW_s[:, :],
                                     start=True, stop=True)
                    nc.vector.tensor_add(z_sb[0:csz, 128 * j:128 * j + 128],
                                         zps[0:csz, :],
                                         skip_sb[0:csz, 128 * j:128 * j + 128])
                    bns = bwp.tile([128, 6], F32, tag="bns")
                    nc.vector.bn_stats(bns[0:csz, :], z_sb[0:csz, 128 * j:128 * j + 128])
                    nc.vector.bn_aggr(mv1[0:csz, 2 * j:2 * j + 2], bns[0:csz, :])
                _ln_alpha(nc, bwp, mv1, N_QT, rs1, eps_s[:, 0:1], nmu=nmu1)

                # MLP per chunk + post-LN stats
                for j, off, csz in _q_tiles():
                    zln = bwp.tile([128, 128], BF16, tag="zln")
                    nc.vector.tensor_scalar(zln[0:csz, :], z_sb[0:csz, 128 * j:128 * j + 128],
                                            nmu1[0:csz, j:j + 1], rs1[0:csz, j:j + 1],
                                            OP.add, OP.mult)
                    trz = tp.tile([128, 128], BF16, tag="trz")
                    nc.tensor.matmul(trz[:, 0:csz], zln[0:csz, :], id16_s[0:csz, 0:csz],
                                     is_transpose=True, start=True, stop=True)
                    zlnT = bwp.tile([128, 128], BF16, tag="zlnT")
                    nc.vector.tensor_copy(zlnT[:, 0:csz], trz[:, 0:csz])
                    hg = bwp.tile([128, 2, 128], BF16, tag="hg")
                    for bi, w1s in ((0, w1a_s), (1, w1b_s)):
                        hps = hp.tile([128, 128], F32, tag=f"hps{bi}")
                        nc.tensor.matmul(hps[0:128, 0:csz], w1s[:, :], zlnT[:, 0:csz],
                                         start=True, stop=True)
                        gb = (b1a_s if bi == 0 else b1b_s)
                        nc.scalar.activation(hg[:, bi, 0:csz], hps[0:128, 0:csz],
                                             AF.Gelu,
                                             bias=(gb[:, 0:1] if has_b1 else 0.0))
                    mps = zp.tile([128, 128], F32, tag="mps")
                    nc.tensor.matmul(mps[0:csz, :], hg[:, 0, 0:csz], w2a_s[:, :],
                                     start=True, stop=False, skip_group_check=True)
                    nc.tensor.matmul(mps[0:csz, :], hg[:, 1, 0:csz], w2b_s[:, :],
                                     start=False, stop=not has_b2,
                                     skip_group_check=True)
                    if has_b2:
                        nc.tensor.matmul(mps[0:csz, :], ones16_s[0:1, 0:csz],
                                         b2_s[0:1, :], start=False, stop=True,
                                         skip_group_check=True)
                    zr2 = bwp.tile([128, 128], F32, tag="zr2")
                    nc.vector.tensor_add(zr2[0:csz, :], mps[0:csz, :],
                                         z_sb[0:csz, 128 * j:128 * j + 128])
                    nc.vector.tensor_copy(z_sb[0:csz, 128 * j:128 * j + 128], zr2[0:csz, :])
                    bns2 = bwp.tile([128, 6], F32, tag="bns2")
                    nc.vector.bn_stats(bns2[0:csz, :], zr2[0:csz, :])
                    nc.vector.bn_aggr(mv2[0:csz, 2 * j:2 * j + 2], bns2[0:csz, :])
                _ln_alpha(nc, bwp, mv2, N_QT, rs2, eps_s[:, 0:1], nmu=nmu2)

                for j, off, csz in _q_tiles():
                    zo = bwp.tile([128, 128], F32, tag="zo")
                    nc.vector.tensor_scalar(zo[0:csz, :], z_sb[0:csz, 128 * j:128 * j + 128],
                                            nmu2[0:csz, j:j + 1], rs2[0:csz, j:j + 1],
                                            OP.add, OP.mult)
                    if has_post:
                        zo2 = bwp.tile([128, 128], F32, tag="zo2")
                        nc.vector.tensor_mul(zo2[0:csz, :], zo[0:csz, :],
                                             pog_s[0:csz, :])
                        nc.vector.tensor_add(zo[0:csz, :], zo2[0:csz, :],
                                             pob_s[0:csz, :])
                    tro = tp.tile([128, 128], F32, tag="tro")
                    nc.tensor.matmul(tro[:, 0:csz], zo[0:csz, :], id32_s[0:csz, 0:csz],
                                     is_transpose=True, start=True, stop=True)
                    nc.vector.tensor_copy(outfm[:, off:off + csz], tro[:, 0:csz])

                nc.sync.dma_start(out=y[:], in_=outfm[:])

    return nc


def _host_prep(inputs):
    f = np.float32
    bf = ml_dtypes.bfloat16
    g = {}
    scale = np.float32(DHEAD ** -0.5)
    wq_e = (np.asarray(inputs["ln_q_g"], f)[:, None] * np.asarray(inputs["wq"], f)) * scale
    bq_e = (np.asarray(inputs["ln_q_b"], f) @ np.asarray(inputs["wq"], f)
            + np.asarray(inputs["bq"], f)) * scale
    wk_e = np.asarray(inputs["ln_k_g"], f)[:, None] * np.asarray(inputs["wk"], f)
    wv_e = np.asarray(inputs["ln_v_g"], f)[:, None] * np.asarray(inputs["wv"], f)
    bv_e = (np.asarray(inputs["ln_v_b"], f) @ np.asarray(inputs["wv"], f)
            + np.asarray(inputs["bv"], f))
    # mean-centering folded into weights: (x - mu) @ W == x @ (W - colsum/D)
    wqc = wq_e - wq_e.sum(0, keepdims=True) / D
    wkc = wk_e - wk_e.sum(0, keepdims=True) / D
    wvc = wv_e - wv_e.sum(0, keepdims=True) / D

    proj_w = np.asarray(inputs["proj_w"], f)
    proj_b_eff = np.asarray(inputs["proj_b"], f) + bv_e @ proj_w

    # broadcast map: z4r rows {0,32,64,96} hold 1/Z for heads 0..3
    zhmA = np.zeros((128, 128), f)
    for i in range(4):
        zhmA[32 * i, 32 * i:32 * i + 32] = 1.0

    pre_g = np.asarray(inputs["pre_g"], f)
    pre_b = np.asarray(inputs["pre_b"], f)
    w1_e = pre_g[:, None] * np.asarray(inputs["mlp_w1"], f)
    b1_e = pre_b @ np.asarray(inputs["mlp_w1"], f) + np.asarray(inputs["mlp_b1"], f)
    w2 = np.asarray(inputs["mlp_w2"], f)
    b2_e = np.asarray(inputs["mlp_b2"], f)
    post_g = np.asarray(inputs["post_g"], f)
    post_b = np.asarray(inputs["post_b"], f)

    g["wqc"] = np.ascontiguousarray(wqc.astype(bf))
    g["wkc"] = np.ascontiguousarray(wkc.astype(bf))
    g["wvc"] = np.ascontiguousarray(wvc.astype(bf))
    g["pjW"] = np.ascontiguousarray(proj_w.astype(bf))
    g["zhmA"] = np.ascontiguousarray(zhmA.astype(bf))
    g["w1a"] = np.ascontiguousarray(w1_e[:, 0:128].astype(bf))
    g["w1b"] = np.ascontiguousarray(w1_e[:, 128:256].astype(bf))
    g["w2a"] = np.ascontiguousarray(w2[0:128].astype(bf))
    g["w2b"] = np.ascontiguousarray(w2[128:256].astype(bf))
    g["id16"] = np.eye(128, dtype=bf)
    g["id32"] = np.eye(128, dtype=f)
    g["ones16"] = np.ones((128, 128), bf)

    flags = {
        "has_bq": bool(np.any(bq_e != 0)),
        "has_b1": bool(np.any(b1_e != 0)),
        "has_b2": bool(np.any(b2_e != 0)),
        "has_post": not (np.allclose(post_g, 1.0) and np.allclose(post_b, 0.0)),
    }
    if flags["has_bq"]:
        g["bqcol"] = np.ascontiguousarray(bq_e[:, None], dtype=f)
    if flags["has_b1"]:
        g["b1acol"] = np.ascontiguousarray(b1_e[0:128, None], dtype=f)
        g["b1bcol"] = np.ascontiguousarray(b1_e[128:256, None], dtype=f)
    if flags["has_b2"]:
        g["b2row"] = np.ascontiguousarray(b2_e[None, :].astype(bf))
    if flags["has_post"]:
        g["pogb"] = np.ascontiguousarray(np.broadcast_to(post_g[None, :], (128, 128)), f)
        g["pobb"] = np.ascontiguousarray(np.broadcast_to(post_b[None, :], (128, 128)), f)
    return g, flags, proj_b_eff


STARTS = [0, 938, 1876, 2813]
LENS = [938, 938, 937, 937]


def _make_in_maps(inputs):
    f = np.float32
    bf = ml_dtypes.bfloat16
    q = np.asarray(inputs["q"], f)
    k = np.asarray(inputs["k"], f)
    v = np.asarray(inputs["v"], f)
    skip = np.asarray(inputs["skip"], f)
    consts, flags, proj_b_eff = _host_prep(inputs)

    in_maps = []
    for c in range(8):
        b, s = c // 4, c % 4
        qfm = np.ascontiguousarray(q[b].transpose(1, 0, 2).reshape(128, NQ_FULL))
        sfm = np.ascontiguousarray(skip[b].transpose(1, 0, 2).reshape(128, NQ_FULL))
        kfm = np.ascontiguousarray(k[b].transpose(1, 0, 2, 3).reshape(128, NK))
        vfm = np.ascontiguousarray(v[b].transpose(1, 0, 2, 3).reshape(128, NK))
        xq = np.zeros((128, TQ), bf)
        xq[:, :LENS[s]] = qfm[:, STARTS[s]:STARTS[s] + LENS[s]].astype(bf)
        sk = np.zeros((128, TQ), f)
        sk[:, :LENS[s]] = sfm[:, STARTS[s]:STARTS[s] + LENS[s]]
        # token-major skip tiles with proj bias folded in
        skip_tm = np.zeros((128, N_QT * 128), f)
        for j in range(N_QT):
            off = j * KT
            tsz = min(KT, TQ - off)
            skip_tm[0:tsz, 128 * j:128 * j + 128] = sk[:, off:off + tsz].T + proj_b_eff[None, :]
        m = {"xq": xq, "xk": kfm.astype(bf), "xv": vfm.astype(bf),
             "xskip": skip_tm}
        m.update(consts)
        in_maps.append(m)
    return in_maps, flags


_CACHE = {}


def kernel(**inputs):
    f = np.float32
    in_maps, flags = _make_in_maps(inputs)

    key = tuple(sorted(flags.items()))
    if key not in _CACHE:
        _CACHE[key] = build_program(flags)
    nc = _CACHE[key]

    _install_compile_patch()
    res = bass_utils.run_bass_kernel_spmd(nc, in_maps, core_ids=list(range(8)))

    full = np.zeros((B, 128, NQ_FULL), f)
    for c in range(8):
        b, s = c // 4, c % 4
        full[b][:, STARTS[s]:STARTS[s] + LENS[s]] = res.results[c]["y"][:, :LENS[s]]
    return np.ascontiguousarray(
        full.reshape(B, 128, N, M).transpose(0, 2, 1, 3))
